# revision 47
# baseline (speedup 1.0000x reference)
"""fp8-DoubleRow revision of the causal cross-attention kernel.

See kernel.py (fp16 shipped version) for the base pipeline.  Differences:
full-512 chunks strictly below the diagonal (j < 4c) are computed in PAIRS:
exp writes float8e4 (bias -ln16), and the PV for a pair is ONE DoubleRow
matmul (2 s-chunks per pass).  Pair denominators fold on the host from raw
fp8 DMAs (ACT pairs batched six-per-DMA via a shared big tile).  c0, each
diagonal full and the tails stay fp16.  The scps pool is 3 deep (3x2 PSUM
banks + 2 ops banks), so DVE-offloaded exps no longer stall the tensor
stream.
"""

from contextlib import ExitStack

import ml_dtypes
import numpy as np

import concourse.bass as bass  # noqa: F401
import concourse.mybir as mybir
import concourse.tile as tile
from concourse import bacc
from concourse.bass_utils import run_bass_kernel_spmd

F32 = mybir.dt.float32
F16 = mybir.dt.float16
F8 = mybir.dt.float8e4
U8 = mybir.dt.uint8
E4M3 = ml_dtypes.float8_e4m3fn

N_CORES = 8
TQ = 512
SC = 128
PGRP = 1024
PV_DEPTH = 3
RS_DEPTH = 5
N_WARM = 13
BT_SLOTS = 6

LN16 = 2.7725887
A8 = 8.0 / float(np.log(2.0))
B8C = 7.0 * 8.0 - A8 * LN16 - 0.5
DVE_PAIRS_PER_HEAD = 6


def _plan8(t, s, fast_start=False):
    ntq, nsc = t // TQ, s // SC
    groups = []
    for c in range(ntq):
        for m in range(0, 4 * c, 2):
            groups.append(("pair", c, m))
        ch = []
        for j in range(4 * c, min(nsc, 4 * (c + 1))):
            ls = max(0, SC * j - TQ * c)
            ch.append((c, j, ls, TQ - ls))

        def pack(chunks):
            cur, off = [], 0
            for (cc, j, ls, w) in chunks:
                bank_used = off % TQ
                if bank_used and bank_used + w > TQ:
                    off += TQ - bank_used
                cur.append((cc, j, ls, w, off))
                off += w
            return ("f16", cur, off)

        if fast_start and c == 0:
            groups.append(pack(ch[0:1]))
            groups.append(pack(ch[1:2]))
            groups.append(pack(ch[2:4]))
        else:
            groups.append(pack(ch[0:2]))
            groups.append(pack(ch[2:4]))
    return groups


def _layout(heads, t, s):
    per_head = []
    slots = []
    for h in range(heads):
        groups = _plan8(t, s, fast_start=(h == 0))
        pair_gis = [i for i, g in enumerate(groups) if g[0] == "pair"]
        lim = pair_gis
        if h == heads - 1:
            lim = [i for i in pair_gis if i < len(groups) - 2]
        k = min(DVE_PAIRS_PER_HEAD, len(lim))
        dve = {lim[(len(lim) * (2 * i + 1)) // (2 * k)] for i in range(k)} \
            if k else set()
        per_head.append((groups, dve))
    bt_fill = 0
    bt_members = []
    n8 = [0]
    n16 = [0]

    def add8(entry):
        slots.append(entry + (n8[0],))
        n8[0] += 1

    def add16(entry):
        slots.append(entry + (n16[0],))
        n16[0] += 1

    for h in range(heads):
        groups, dve = per_head[h]
        for gi, g in enumerate(groups):
            if g[0] == "pair":
                if gi in dve:
                    add8(("dvepair", h, g[1], g[2]))
                else:
                    bt_members.append((h, g[1], g[2], bt_fill))
                    bt_fill += 1
                    if bt_fill == BT_SLOTS:
                        add8(("bigtile", bt_members))
                        bt_members, bt_fill = [], 0
            else:
                pass  # f16 groups row-sum on the vector engine (no raw DMA)
    if bt_members:
        add8(("bigtile", bt_members))
    return per_head, slots, n8[0], n16[0]


def build_program(heads_per_core=4, t=2048, s=2048, d=128, trivial_mask=True):
    assert t % TQ == 0 and s % SC == 0 and d == 128
    ntq, nsc = t // TQ, s // SC
    per_head, slots, n8, n16 = _layout(heads_per_core, t, s)
    QCOL, KCOL, VCOL = 0, t, t + s

    nc = bacc.Bacc(
        "TRN2", target_bir_lowering=False, debug=False, enable_asserts=False
    )
    qkv_d = nc.dram_tensor(
        "qkv", [heads_per_core, d, t + s + nsc * d], F16, kind="ExternalInput"
    ).ap()
    v8_d = nc.dram_tensor(
        "v8", [heads_per_core, d, s], F8, kind="ExternalInput"
    ).ap()
    pad_d = nc.dram_tensor("padexp", [SC, nsc], F32, kind="ExternalInput").ap()
    outT_d = nc.dram_tensor(
        "outT", [heads_per_core, d, t], F16, kind="ExternalOutput"
    ).ap()
    acc_d = nc.dram_tensor(
        "accs", [heads_per_core, SC, t], F16, kind="ExternalOutput"
    ).ap()
    raw8_d = nc.dram_tensor(
        "raw8", [max(n8, 1), SC, BT_SLOTS * PGRP], F8, kind="ExternalOutput"
    ).ap()


    n_pv_of_c = [2 * c + 4 if c else 4 for c in range(ntq)]

    with tile.TileContext(nc) as tc, ExitStack() as ctx:
        inp = ctx.enter_context(tc.tile_pool(name="inp", bufs=1))
        xp = ctx.enter_context(tc.tile_pool(name="xp", bufs=12))
        btp = ctx.enter_context(tc.tile_pool(name="btp", bufs=2))
        accp = ctx.enter_context(tc.tile_pool(name="accp", bufs=2))
        osbp = ctx.enter_context(tc.tile_pool(name="osbp", bufs=4))
        padp = ctx.enter_context(tc.tile_pool(name="padp", bufs=1))
        cbp = ctx.enter_context(tc.tile_pool(name="cbp", bufs=1))
        scps = ctx.enter_context(tc.tile_pool(name="scps", bufs=3, space="PSUM"))
        ops_ = ctx.enter_context(tc.tile_pool(name="ops", bufs=2, space="PSUM"))

        qkvs = [
            inp.tile([d, t + s + nsc * d], F16, tag=f"qkv{h}", name=f"qkv{h}")
            for h in range(heads_per_core)
        ]
        v8s = [
            inp.tile([d, s], F8, tag=f"v8{h}", name=f"v8{h}")
            for h in range(heads_per_core)
        ]

        def d0(col0, col1):
            nc.sync.dma_start(
                out=qkvs[0][:, col0:col1], in_=qkv_d[0][:, col0:col1]
            )

        d0(KCOL, KCOL + SC)
        d0(QCOL, QCOL + 2 * SC)
        d0(QCOL + 2 * SC, QCOL + TQ)
        d0(KCOL + SC, KCOL + TQ)
        d0(QCOL + TQ, QCOL + 2 * TQ)
        d0(KCOL + TQ, KCOL + 2 * TQ)
        # v0a before the 256KB v8[0]: the fp16 v slice feeds c0's PVs at
        # ~13us while v8 isn't read until the first c1 pair PV (~20us), and
        # issuing the big v8 transfer first made it compete with the
        # critical k/q slices for DMA engines (first exp 13.0us vs ~10.8).
        nc.scalar.dma_start(
            out=qkvs[0][:, VCOL : VCOL + TQ], in_=qkv_d[0][:, VCOL : VCOL + TQ]
        )
        nc.scalar.dma_start(out=v8s[0][:], in_=v8_d[0][:])
        d0(QCOL + 2 * TQ, QCOL + t)
        d0(KCOL + 2 * TQ, KCOL + s)
        d0(VCOL + TQ, VCOL + nsc * d)
        padexp = None
        if not trivial_mask:
            padexp = padp.tile([SC, nsc], F32, name="padexp")
            nc.sync.dma_start(out=padexp[:], in_=pad_d[:])
        for h in range(1, heads_per_core):
            nc.sync.dma_start(out=qkvs[h][:], in_=qkv_d[h][:])
            nc.sync.dma_start(out=v8s[h][:], in_=v8_d[h][:])

        expb = cbp.tile([SC, 1], F32, name="expb")
        nc.vector.memset(expb[:], -LN16)

        wl = nc.const_aps.tensor(1.0, (d, SC), mybir.dt.bfloat16)
        wr = nc.const_aps.tensor(1.0, (d, 2 * SC), mybir.dt.bfloat16)
        warm_ps = scps.tile([SC, PGRP], F32, tag="sc", name="warm_ps")
        for _ in range(N_WARM):
            nc.tensor.matmul(
                out=warm_ps[:, 0 : 2 * SC], lhsT=wl, rhs=wr,
                start=True, stop=True,
            )

        ops_t, acc_t, first_full = {}, {}, {}
        pv_seen, rs_seen, flushed = {}, {}, {}
        acc_head = {}
        for h in range(heads_per_core):
            for c in range(ntq):
                ops_t[h, c] = acc_t[h, c] = first_full[h, c] = None
                pv_seen[h, c] = rs_seen[h, c] = 0
            acc_head[h] = None
            flushed[h] = 0

        def acc_slice(h, c):
            if acc_head[h] is None:
                acc_head[h] = accp.tile([SC, t], F16, tag="acc", name="acc")
            return acc_head[h][:, TQ * c : TQ * (c + 1)]

        def pv_done(h, c):
            osb = osbp.tile([d, TQ], F16, tag="osb", name="osb")
            nc.vector.tensor_copy(osb[:], ops_t[h, c][:])
            nc.sync.dma_start(
                out=outT_d[h][:, TQ * c : TQ * (c + 1)], in_=osb[:]
            )

        def emit_pv_pair(h, c, j0, ext8):
            if ops_t[h, c] is None:
                ops_t[h, c] = ops_.tile([d, TQ], F32, tag="ops", name="ops")
            pv_seen[h, c] += 1
            nc.tensor.matmul(
                out=ops_t[h, c][:, 0:TQ],
                lhsT=v8s[h][:, SC * j0 : SC * (j0 + 2)].rearrange(
                    "p (two m) -> p two m", two=2
                ),
                rhs=ext8.rearrange("p (two n) -> p two n", two=2),
                start=(pv_seen[h, c] == 1),
                stop=(pv_seen[h, c] == n_pv_of_c[c]),
                perf_mode=mybir.MatmulPerfMode.DoubleRow,
            )
            if pv_seen[h, c] == n_pv_of_c[c]:
                pv_done(h, c)

        def emit_pv_f16(h, c, j, ls, w, sl):
            qkv = qkvs[h]
            if ops_t[h, c] is None:
                ops_t[h, c] = ops_.tile([d, TQ], F32, tag="ops", name="ops")
            pv_seen[h, c] += 1
            nc.tensor.matmul(
                out=ops_t[h, c][:, ls:TQ],
                lhsT=qkv[:, VCOL + SC * j : VCOL + SC * (j + 1)],
                rhs=sl,
                start=(pv_seen[h, c] == 1),
                stop=(pv_seen[h, c] == n_pv_of_c[c]),
            )
            if pv_seen[h, c] == n_pv_of_c[c]:
                pv_done(h, c)

        deferred = []
        defer_until = {}

        def pv_round(h, grp, ext, round_):
            flush = [a for a in deferred if round_ >= defer_until.get(a[0], 0)][:3]
            for a in flush:
                deferred.remove(a)
            for a in flush:
                if a[1] == "pair":
                    emit_pv_pair(a[0], a[2], a[3], a[4])
                else:
                    emit_pv_f16(a[0], *a[2:])
            if grp[0] == "pair":
                _, c, j0 = grp
                if (
                    c <= 1 and ops_t[h, c] is None
                    and round_ < defer_until.get(h, 0)
                ):
                    deferred.append((h, "pair", c, j0, ext))
                else:
                    emit_pv_pair(h, c, j0, ext)
            else:
                for (c, j, ls, w, off) in grp[1]:
                    sl = ext[:, off : off + w]
                    if (
                        c <= 1 and ops_t[h, c] is None
                        and round_ < defer_until.get(h, 0)
                    ):
                        deferred.append((h, "f16", c, j, ls, w, sl))
                    else:
                        emit_pv_f16(h, c, j, ls, w, sl)

        def rs_round(h, grp, ext):
            if grp[0] == "pair":
                return
            for (c, j, ls, w, off) in grp[1]:
                sl = ext[:, off : off + w]
                if acc_t[h, c] is not None:
                    nc.vector.tensor_add(
                        acc_t[h, c][:, ls:TQ], acc_t[h, c][:, ls:TQ], sl
                    )
                elif first_full[h, c] is not None:
                    fls, fsl = first_full[h, c]
                    acc_t[h, c] = acc_slice(h, c)
                    if fls == 0 and ls == 0 and w == TQ:
                        nc.vector.tensor_add(acc_t[h, c][:], fsl, sl)
                    else:
                        nc.vector.tensor_copy(acc_t[h, c][:, fls:TQ], fsl)
                        nc.vector.tensor_add(
                            acc_t[h, c][:, ls:TQ], acc_t[h, c][:, ls:TQ], sl
                        )
                else:
                    assert ls == 0, (h, c, j, ls)
                    first_full[h, c] = (ls, sl)
                rs_seen[h, c] += 1
                if rs_seen[h, c] == 4:
                    if acc_t[h, c] is None:
                        acc_t[h, c] = acc_slice(h, c)
                        fls, fsl = first_full[h, c]
                        nc.vector.tensor_copy(acc_t[h, c][:, fls:TQ], fsl)
                    flushed[h] += 1
                    if flushed[h] == ntq:
                        nc.sync.dma_start(out=acc_d[h], in_=acc_head[h][:])

        slot_i = [0]
        bt_tile = [None]
        bt_count = [0]

        def bigtile_slot():
            """Returns (slice, flush): the flush closure must be called AFTER
            the slice's writer is emitted (emitting the DMA first would ship
            the final slice as unwritten garbage -> NaN denominators)."""
            if bt_tile[0] is None:
                bt_tile[0] = btp.tile(
                    [SC, BT_SLOTS * PGRP], F8, tag="bt", name="bt"
                )
            i = bt_count[0]
            bt_count[0] += 1
            sl = bt_tile[0][:, PGRP * i : PGRP * (i + 1)]
            flush = None
            if bt_count[0] == BT_SLOTS:
                tile_ref = bt_tile[0]
                idx8 = slots[slot_i[0]][-1]
                slot_i[0] += 1
                bt_tile[0] = None
                bt_count[0] = 0

                def flush():
                    nc.sync.dma_start(out=raw8_d[idx8], in_=tile_ref[:])
            return sl, flush

        pv_pipe, rs_pipe = [], []
        round_ = 0
        for h in range(heads_per_core):
            qkv = qkvs[h]
            if h > 0:
                defer_until[h] = round_ + 8
            groups, dve = per_head[h]
            for gi, grp in enumerate(groups):
                sct = scps.tile([SC, PGRP], F32, tag="sc", name=f"s{h}g{gi}")
                if grp[0] == "pair":
                    _, c, j0 = grp
                    for half in (0, 1):
                        nc.tensor.matmul(
                            out=sct[:, TQ * half : TQ * (half + 1)],
                            lhsT=qkv[:, KCOL + SC * (j0 + half) :
                                     KCOL + SC * (j0 + half + 1)],
                            rhs=qkv[:, QCOL + TQ * c : QCOL + TQ * (c + 1)],
                            start=True,
                            stop=True,
                        )
                    if gi in dve:
                        ext8 = xp.tile([SC, PGRP], F8, tag="e8", name="ext8")
                        nc.vector.tensor_scalar(
                            out=ext8[:].bitcast(U8),
                            in0=sct[:],
                            scalar1=A8,
                            scalar2=B8C,
                            op0=mybir.AluOpType.mult,
                            op1=mybir.AluOpType.add,
                        )
                        idx8 = slots[slot_i[0]][-1]
                        nc.sync.dma_start(
                            out=raw8_d[idx8][:, 0:PGRP], in_=ext8[:]
                        )
                        slot_i[0] += 1
                        exv = ext8[:]
                    else:
                        sl, flush = bigtile_slot()
                        nc.scalar.activation(
                            out=sl,
                            in_=sct[:],
                            func=mybir.ActivationFunctionType.Exp,
                            bias=expb[:],
                        )
                        exv = sl
                    if padexp is not None:
                        for half in (0, 1):
                            nc.vector.tensor_scalar(
                                out=exv[:, TQ * half : TQ * (half + 1)],
                                in0=exv[:, TQ * half : TQ * (half + 1)],
                                scalar1=padexp[:, j0 + half : j0 + half + 1],
                                scalar2=None,
                                op0=mybir.AluOpType.mult,
                            )
                    if grp[0] == "pair" and gi not in dve and flush is not None:
                        flush()
                    pv_pipe.append((h, grp, exv))
                    rs_pipe.append((h, grp, exv))
                else:
                    _, ch, used = grp
                    for (c, j, ls, w, off) in ch:
                        nc.tensor.matmul(
                            out=sct[:, off : off + w],
                            lhsT=qkv[:, KCOL + SC * j : KCOL + SC * (j + 1)],
                            rhs=qkv[:, QCOL + TQ * c + ls : QCOL + TQ * (c + 1)],
                            start=True,
                            stop=True,
                        )
                    ext = xp.tile([SC, PGRP], F16, tag="ex", name="ext")
                    nc.scalar.activation(
                        out=ext[:, 0:used],
                        in_=sct[:, 0:used],
                        func=mybir.ActivationFunctionType.Exp,
                        bias=expb[:],
                    )
                    for (c, j, ls, w, off) in ch:
                        if SC * j >= TQ * c:
                            nc.gpsimd.affine_select(
                                out=ext[:, off : off + SC],
                                in_=ext[:, off : off + SC],
                                pattern=[[1, SC]],
                                compare_op=mybir.AluOpType.is_ge,
                                fill=0.0,
                                base=0,
                                channel_multiplier=-1,
                            )
                        if padexp is not None:
                            nc.vector.tensor_scalar(
                                out=ext[:, off : off + w],
                                in0=ext[:, off : off + w],
                                scalar1=padexp[:, j : j + 1],
                                scalar2=None,
                                op0=mybir.AluOpType.mult,
                            )
                    pv_pipe.append((h, grp, ext))
                    rs_pipe.append((h, grp, ext))
                round_ += 1
                if len(pv_pipe) > PV_DEPTH:
                    pv_round(*pv_pipe.pop(0), round_)
                rs_depth = PV_DEPTH if h == heads_per_core - 1 else RS_DEPTH
                while len(rs_pipe) > rs_depth:
                    rs_round(*rs_pipe.pop(0))
        while pv_pipe or rs_pipe:
            round_ += 1
            if pv_pipe:
                pv_round(*pv_pipe.pop(0), round_)
            if rs_pipe:
                rs_round(*rs_pipe.pop(0))
        for a in deferred:
            if a[1] == "pair":
                emit_pv_pair(a[0], a[2], a[3], a[4])
            else:
                emit_pv_f16(a[0], *a[2:])
        if bt_tile[0] is not None:
            idx8 = slots[slot_i[0]][-1]
            nc.sync.dma_start(
                out=raw8_d[idx8][:, 0 : PGRP * bt_count[0]],
                in_=bt_tile[0][:, 0 : PGRP * bt_count[0]],
            )
            slot_i[0] += 1
            bt_tile[0] = None
        assert slot_i[0] == len(slots), (slot_i[0], len(slots))

    nc.compile()
    return nc


def make_in_maps(q, kv, attention_mask):
    b, t, h, d = q.shape
    s = kv.shape[1]
    nsc = s // SC
    hpc = (b * h) // N_CORES
    scale = np.float32(1.0 / np.sqrt(d))
    q = np.asarray(q, dtype=np.float32)
    k = np.asarray(kv[:, :, 0], dtype=np.float32)
    v = np.asarray(kv[:, :, 1], dtype=np.float32)
    mask = np.asarray(attention_mask)
    pairs_per_b = h // hpc

    in_maps = []
    for core in range(N_CORES):
        bb = core // pairs_per_b
        h0 = (core % pairs_per_b) * hpc
        qT = np.ascontiguousarray(
            q[bb, :, h0 : h0 + hpc, :].transpose(1, 2, 0) * scale
        ).astype(np.float16)
        kT = np.ascontiguousarray(
            k[bb, :, h0 : h0 + hpc, :].transpose(1, 2, 0)
        ).astype(np.float16)
        vv = (
            v[bb, :, h0 : h0 + hpc, :]
            .transpose(1, 0, 2)
            .reshape(hpc, nsc, SC, d)
            .transpose(0, 2, 1, 3)
            .reshape(hpc, SC, nsc * d)
        ).astype(np.float16)
        qkv = np.ascontiguousarray(np.concatenate([qT, kT, vv], axis=2))
        v8 = np.ascontiguousarray(vv.astype(E4M3)).view(np.uint8)
        pad = np.where(mask[bb], np.float32(1.0), np.float32(0.0)).astype(
            np.float32
        )
        padexp = np.ascontiguousarray(pad.reshape(nsc, SC).T)
        in_maps.append({"qkv": qkv, "v8": v8, "padexp": padexp})
    return in_maps


def assemble_output(results, b, t, h, d):
    hpc = (b * h) // N_CORES
    pairs_per_b = h // hpc
    per_head, slots, n8, n16 = _layout(hpc, t, t)
    out = np.empty((b, t, h, d), dtype=np.float32)
    for core, res in enumerate(results):
        bb = core // pairs_per_b
        h0 = (core % pairs_per_b) * hpc
        outT = res["outT"].astype(np.float32)
        accs = res["accs"]  # [hpc, SC, t]
        raw8 = np.asarray(res["raw8"]).view(E4M3)
        denom = accs.astype(np.float32).sum(axis=1)
        for slot in slots:
            if slot[0] == "bigtile":
                _, members, si = slot
                for (hh, c, j0, i) in members:
                    seg = raw8[si][:, PGRP * i : PGRP * (i + 1)]
                    ps = seg.astype(np.float32).sum(axis=0)
                    denom[hh, TQ * c : TQ * (c + 1)] += ps[0:TQ] + ps[TQ:]
            elif slot[0] == "dvepair":
                _, hh, c, j0, si = slot
                seg = raw8[si][:, 0:PGRP]
                ps = seg.astype(np.float32).sum(axis=0)
                denom[hh, TQ * c : TQ * (c + 1)] += ps[0:TQ] + ps[TQ:]
            else:
                pass
        norm = (outT / denom[:, None, :]).transpose(0, 2, 1)
        out[bb, :, h0 : h0 + hpc, :] = norm.transpose(1, 0, 2)
    return out


_CACHE = {}


def _get_program(trivial_mask):
    key = bool(trivial_mask)
    if key not in _CACHE:
        _CACHE[key] = build_program(trivial_mask=key)
    return _CACHE[key]


def run(q, kv, attention_mask, trace=False):
    b, t, h, d = q.shape
    trivial = bool(np.asarray(attention_mask).all())
    nc = _get_program(trivial)
    in_maps = make_in_maps(q, kv, attention_mask)
    br = run_bass_kernel_spmd(nc, in_maps, list(range(N_CORES)), trace=trace)
    return assemble_output(br.results, b, t, h, d), br


def kernel(q, kv, attention_mask):
    out, _ = run(q, kv, attention_mask)
    return out
